# revision 14
# baseline (speedup 1.0000x reference)
"""KAN layer kernel for TRN2, 8-core SPMD.

Math: out[b,o] = sum_{i,k} relu(x[b,i]*w1[o,i,k] + b1[o,i,k]) * w2[o,i,k] / 32 + b2[o]
With b1 == 0 (guaranteed by the generator) the relu factorizes by the sign
of x:  relu(x*w1) = max(x,0)*max(w1,0) + min(x,0)*min(w1,0), so the layer
is exactly two matmuls with preprocessed weights:
    Ap[i,o] = sum_k max(w1,0)*w2      Am[i,o] = sum_k min(w1,0)*w2
    out = (relu(x) @ Ap + min(x,0) @ Am) * S + b2        (S = 1/32)
The k-sum is done as pair sums on DVE/Pool; the final pair-add is absorbed
into the matmul (two lhsT pair-slabs per tile accumulate into one PSUM).

Sharding: 4 batch groups x 2 dout groups (core = bi*2 + oj).
All wire traffic is bf16 (packed on host, output upcast on host);
accumulation stays f32 in PSUM.
"""

import numpy as np

B, DIN, DOUT, K = 2048, 256, 256, 4
N_CORES = 8
BG, OG = 4, 2                      # batch groups x dout groups
BS, OS = B // BG, DOUT // OG       # 512 batch rows, 128 dout cols per core
NT = DIN // 128                    # din partition tiles (2)
NC, CB = 2, 256                    # batch chunks per core
SCALE = 1.0 / np.sqrt(((DOUT + DIN) / 2) * K)   # 1/32
NWARM = 19                         # PE p-state warmup matmuls

_CACHE = {}


def _build_nc():
    if "nc" in _CACHE:
        return _CACHE["nc"]
    import concourse.bacc as bacc
    import concourse.tile as tile
    from concourse import mybir

    f32 = mybir.dt.float32
    bf16 = mybir.dt.bfloat16
    AF = mybir.ActivationFunctionType
    OP = mybir.AluOpType

    nc = bacc.Bacc("TRN2", target_bir_lowering=False, debug=False,
                   num_devices=N_CORES)
    wpk = nc.dram_tensor("wpk", [128, NT, 2, K, OS], bf16, kind="ExternalInput")
    xpk = nc.dram_tensor("xpk", [128, NC, NT, CB], bf16, kind="ExternalInput")
    b2s = nc.dram_tensor("b2s", [OS, 1], f32, kind="ExternalInput")
    outt = nc.dram_tensor("outt", [OS, BS], bf16, kind="ExternalOutput")

    with tile.TileContext(nc) as tc:
        with (
            tc.tile_pool(name="io", bufs=1) as io,
            tc.tile_pool(name="wk", bufs=1) as wk,
            tc.tile_pool(name="pp", bufs=1, space="PSUM") as pp,
        ):
            wsb = io.tile([128, NT, 2, K, OS], bf16)
            xsb = io.tile([128, NC, NT, CB], bf16)
            b2sb = io.tile([OS, 1], f32)

            # PE warmup operands (zeros; results never read)
            wm = wk.tile([128, 128], bf16)
            wr = wk.tile([128, 256], bf16)
            nc.vector.memset(wm, 0.0)
            nc.vector.memset(wr, 0.0)

            # input DMAs, all on the SP queue: w tiles first (prep is the
            # long pole), then x chunks, then the tiny bias; more/smaller
            # DMAs lose: HWDGE issue serialization is 650ns per DMA
            for t in range(NT):
                nc.sync.dma_start(out=wsb[:, t], in_=wpk[:, t])
            for c in range(NC):
                nc.sync.dma_start(out=xsb[:, c], in_=xpk[:, c])
            nc.sync.dma_start(out=b2sb, in_=b2s[:, :])

            # PE p-state warmup: back-to-back dummy matmuls keep the tensor
            # engine continuously busy so the real matmuls see a hot clock
            pw = pp.tile([128, 256], f32)
            for _ in range(NWARM):
                nc.tensor.matmul(pw, lhsT=wm, rhs=wr, start=True, stop=True)

            # weight prep: mw/nw = max/min(w1,0) on DVE (fast 4x tensor_scalar),
            # products on DVE, k pair-sums split DVE/Pool
            mw, nw, apt, amt = [None] * NT, [None] * NT, [None] * NT, [None] * NT
            ap2, am2 = [None] * NT, [None] * NT
            xp, xn = [None] * NC, [None] * NC
            for t in range(NT):
                mw[t] = wk.tile([128, K, OS], bf16, name=f"mw{t}")
                nw[t] = wk.tile([128, K, OS], bf16, name=f"nw{t}")
                apt[t] = wk.tile([128, K, OS], bf16, name=f"apt{t}")
                amt[t] = wk.tile([128, K, OS], bf16, name=f"amt{t}")
                ap2[t] = wk.tile([128, 2, OS], bf16, name=f"ap2{t}")
                am2[t] = wk.tile([128, 2, OS], bf16, name=f"am2{t}")
            for c in range(NC):
                xp[c] = wk.tile([128, NT, CB], bf16, name=f"xp{c}")
                xn[c] = wk.tile([128, NT, CB], bf16, name=f"xn{c}")

            # DVE stream (in program order)
            nc.vector.tensor_scalar(mw[0], wsb[:, 0, 0], 0.0, None, op0=OP.max)
            nc.vector.tensor_scalar(nw[0], wsb[:, 0, 0], 0.0, None, op0=OP.min)
            nc.vector.tensor_tensor(apt[0], mw[0], wsb[:, 0, 1], op=OP.mult)
            nc.vector.tensor_tensor(amt[0], nw[0], wsb[:, 0, 1], op=OP.mult)
            nc.vector.tensor_scalar(mw[1], wsb[:, 1, 0], 0.0, None, op0=OP.max)
            nc.vector.tensor_scalar(nw[1], wsb[:, 1, 0], 0.0, None, op0=OP.min)
            nc.vector.tensor_tensor(apt[1], mw[1], wsb[:, 1, 1], op=OP.mult)
            nc.vector.tensor_scalar(xn[0], xsb[:, 0], 0.0, None, op0=OP.min)
            nc.vector.tensor_tensor(amt[1], nw[1], wsb[:, 1, 1], op=OP.mult)
            nc.vector.tensor_scalar(xn[1], xsb[:, 1], 0.0, None, op0=OP.min)
            nc.vector.tensor_tensor(am2[1], amt[1][:, 0:2, :], amt[1][:, 2:4, :],
                                    op=OP.add)
            # Pool stream: tile0 pair-adds + tile1's ap pair-add
            nc.gpsimd.tensor_tensor(ap2[0], apt[0][:, 0:2, :], apt[0][:, 2:4, :],
                                    op=OP.add)
            nc.gpsimd.tensor_tensor(am2[0], amt[0][:, 0:2, :], amt[0][:, 2:4, :],
                                    op=OP.add)
            nc.gpsimd.tensor_tensor(ap2[1], apt[1][:, 0:2, :], apt[1][:, 2:4, :],
                                    op=OP.add)
            # ACT stream: relu(x) per chunk, then the epilogue
            for c in range(NC):
                nc.scalar.activation(xp[c], xsb[:, c], AF.Relu)

            # matmuls: one f32 PSUM tile (own bank) per chunk. Emission order
            # is runnable-first: everything gated only by tile-0 prep runs
            # before anything needing tile-1 prep (which lands last), and
            # chunk 0 closes before chunk 1 so its epilogue can start early.
            ps = [pp.tile([128, CB], f32, name=f"ps{c}") for c in range(NC)]
            with tc.high_priority():
                for c in range(NC):
                    for p in range(2):
                        nc.tensor.matmul(ps[c], lhsT=ap2[0][:, p, :],
                                         rhs=xp[c][:, 0, :],
                                         start=(p == 0), stop=False)
                    for p in range(2):
                        nc.tensor.matmul(ps[c], lhsT=am2[0][:, p, :],
                                         rhs=xn[c][:, 0, :],
                                         start=False, stop=False)
            for c in range(NC):
                for p in range(2):
                    nc.tensor.matmul(ps[c], lhsT=ap2[1][:, p, :],
                                     rhs=xp[c][:, 1, :],
                                     start=False, stop=False)
                for p in range(2):
                    nc.tensor.matmul(ps[c], lhsT=am2[1][:, p, :],
                                     rhs=xn[c][:, 1, :],
                                     start=False, stop=(p == 1))

            # fused epilogue: out = S * psum + b2, bf16 — chunk 0 on ACT,
            # chunk 1 in parallel on DVE — then one store
            osb = wk.tile([128, BS], bf16)
            nc.scalar.activation(osb[:, 0:CB], ps[0],
                                 AF.Identity, bias=b2sb, scale=float(SCALE))
            nc.vector.tensor_scalar(osb[:, CB:BS], ps[1], float(SCALE),
                                    b2sb[:, 0:1], op0=OP.mult, op1=OP.add)
            nc.sync.dma_start(out=outt[:, :], in_=osb)

    nc.compile()
    _CACHE["nc"] = nc
    return nc


def _kan_numpy(x, w1, b1, w2, b2):
    # exact fallback, chunked over batch to bound memory
    out = np.empty((x.shape[0], w1.shape[0]), dtype=np.float32)
    d = (w1.shape[0] + w1.shape[1]) / 2
    s = 1.0 / np.sqrt(d * w1.shape[2])
    for lo in range(0, x.shape[0], 128):
        hi = min(lo + 128, x.shape[0])
        h = x[lo:hi, None, :, None] * w1[None] + b1[None]
        np.maximum(h, 0.0, out=h)
        out[lo:hi] = np.einsum("boik,oik->bo", h, w2) * s
    return out + b2[None, :]


def kernel(x, w1, b1, w2, b2):
    x = np.ascontiguousarray(x, dtype=np.float32)
    w1 = np.asarray(w1, dtype=np.float32)
    b1 = np.asarray(b1, dtype=np.float32)
    w2 = np.asarray(w2, dtype=np.float32)
    b2 = np.asarray(b2, dtype=np.float32)

    if x.shape != (B, DIN) or w1.shape != (DOUT, DIN, K) or np.any(b1):
        return _kan_numpy(x, w1, b1, w2, b2)

    import ml_dtypes
    from concourse.bass_utils import run_bass_kernel_spmd

    bf16 = ml_dtypes.bfloat16
    nc = _build_nc()

    xT = x.T                                      # (DIN, B)
    w1T = w1.transpose(1, 2, 0)                   # (DIN, K, DOUT)
    w2T = w2.transpose(1, 2, 0)

    in_maps = []
    for core in range(N_CORES):
        bi, oj = divmod(core, OG)
        wpk = np.empty((128, NT, 2, K, OS), dtype=bf16)
        xpk = np.empty((128, NC, NT, CB), dtype=bf16)
        osl = slice(oj * OS, (oj + 1) * OS)
        for t in range(NT):
            isl = slice(t * 128, (t + 1) * 128)
            wpk[:, t, 0] = w1T[isl, :, osl]
            wpk[:, t, 1] = w2T[isl, :, osl]
            for c in range(NC):
                bsl = slice(bi * BS + c * CB, bi * BS + (c + 1) * CB)
                xpk[:, c, t, :] = xT[isl, bsl]
        in_maps.append({
            "wpk": wpk,
            "xpk": xpk,
            "b2s": np.ascontiguousarray(b2[osl]).reshape(OS, 1),
        })

    res = run_bass_kernel_spmd(nc, in_maps, core_ids=list(range(N_CORES)))

    out = np.empty((B, DOUT), dtype=np.float32)
    for core in range(N_CORES):
        bi, oj = divmod(core, OG)
        ot = np.asarray(res.results[core]["outt"]).astype(np.float32)
        out[bi * BS:(bi + 1) * BS, oj * OS:(oj + 1) * OS] = ot.T
    return out


# revision 15
# speedup vs baseline: 1.0037x; 1.0037x over previous
"""KAN layer kernel for TRN2, 8-core SPMD.

Math: out[b,o] = sum_{i,k} relu(x[b,i]*w1[o,i,k] + b1[o,i,k]) * w2[o,i,k] / 32 + b2[o]
With b1 == 0 (guaranteed by the generator) the relu factorizes by the sign
of x:  relu(x*w1) = max(x,0)*max(w1,0) + min(x,0)*min(w1,0), so the layer
is exactly two matmuls with preprocessed weights:
    Ap[i,o] = sum_k max(w1,0)*w2      Am[i,o] = sum_k min(w1,0)*w2
    out = (relu(x) @ Ap + min(x,0) @ Am) * S + b2        (S = 1/32)
The k-sum is done as pair sums on DVE/Pool; the final pair-add is absorbed
into the matmul (two lhsT pair-slabs per tile accumulate into one PSUM).

Sharding: 4 batch groups x 2 dout groups (core = bi*2 + oj).
All wire traffic is bf16 (packed on host, output upcast on host);
accumulation stays f32 in PSUM.
"""

import numpy as np

B, DIN, DOUT, K = 2048, 256, 256, 4
N_CORES = 8
BG, OG = 4, 2                      # batch groups x dout groups
BS, OS = B // BG, DOUT // OG       # 512 batch rows, 128 dout cols per core
NT = DIN // 128                    # din partition tiles (2)
NC, CB = 2, 256                    # batch chunks per core
SCALE = 1.0 / np.sqrt(((DOUT + DIN) / 2) * K)   # 1/32
NWARM = 15                         # PE p-state warmup matmuls

_CACHE = {}


def _build_nc():
    if "nc" in _CACHE:
        return _CACHE["nc"]
    import concourse.bacc as bacc
    import concourse.tile as tile
    from concourse import mybir

    f32 = mybir.dt.float32
    bf16 = mybir.dt.bfloat16
    AF = mybir.ActivationFunctionType
    OP = mybir.AluOpType

    nc = bacc.Bacc("TRN2", target_bir_lowering=False, debug=False,
                   num_devices=N_CORES)
    wpk = nc.dram_tensor("wpk", [128, NT, 2, K, OS], bf16, kind="ExternalInput")
    xpk = nc.dram_tensor("xpk", [128, NC, NT, CB], bf16, kind="ExternalInput")
    b2s = nc.dram_tensor("b2s", [OS, 1], f32, kind="ExternalInput")
    outt = nc.dram_tensor("outt", [OS, BS], bf16, kind="ExternalOutput")

    with tile.TileContext(nc) as tc:
        with (
            tc.tile_pool(name="io", bufs=1) as io,
            tc.tile_pool(name="wk", bufs=1) as wk,
            tc.tile_pool(name="pp", bufs=1, space="PSUM") as pp,
        ):
            wsb = io.tile([128, NT, 2, K, OS], bf16)
            xsb = io.tile([128, NC, NT, CB], bf16)
            b2sb = io.tile([OS, 1], f32)

            # PE warmup operands (zeros; results never read)
            wm = wk.tile([128, 128], bf16)
            wr = wk.tile([128, 256], bf16)
            nc.vector.memset(wm, 0.0)
            nc.vector.memset(wr, 0.0)

            # input DMAs, all on the SP queue: w tiles first (prep is the
            # long pole), then x chunks, then the tiny bias; more/smaller
            # DMAs lose: HWDGE issue serialization is 650ns per DMA
            for t in range(NT):
                nc.sync.dma_start(out=wsb[:, t], in_=wpk[:, t])
            for c in range(NC):
                nc.sync.dma_start(out=xsb[:, c], in_=xpk[:, c])
            nc.sync.dma_start(out=b2sb, in_=b2s[:, :])

            # PE p-state warmup: back-to-back dummy matmuls keep the tensor
            # engine continuously busy so the real matmuls see a hot clock
            pw = pp.tile([128, 256], f32)
            for _ in range(NWARM):
                nc.tensor.matmul(pw, lhsT=wm, rhs=wr, start=True, stop=True)

            # weight prep: mw/nw = max/min(w1,0) on DVE (fast 4x tensor_scalar),
            # products on DVE, k pair-sums split DVE/Pool
            mw, nw, apt, amt = [None] * NT, [None] * NT, [None] * NT, [None] * NT
            ap2, am2 = [None] * NT, [None] * NT
            xp, xn = [None] * NC, [None] * NC
            for t in range(NT):
                mw[t] = wk.tile([128, K, OS], bf16, name=f"mw{t}")
                nw[t] = wk.tile([128, K, OS], bf16, name=f"nw{t}")
                apt[t] = wk.tile([128, K, OS], bf16, name=f"apt{t}")
                amt[t] = wk.tile([128, K, OS], bf16, name=f"amt{t}")
                ap2[t] = wk.tile([128, 2, OS], bf16, name=f"ap2{t}")
                am2[t] = wk.tile([128, 2, OS], bf16, name=f"am2{t}")
            for c in range(NC):
                xp[c] = wk.tile([128, NT, CB], bf16, name=f"xp{c}")
                xn[c] = wk.tile([128, NT, CB], bf16, name=f"xn{c}")

            # DVE stream (in program order)
            nc.vector.tensor_scalar(mw[0], wsb[:, 0, 0], 0.0, None, op0=OP.max)
            nc.vector.tensor_scalar(nw[0], wsb[:, 0, 0], 0.0, None, op0=OP.min)
            nc.vector.tensor_tensor(apt[0], mw[0], wsb[:, 0, 1], op=OP.mult)
            nc.vector.tensor_tensor(amt[0], nw[0], wsb[:, 0, 1], op=OP.mult)
            nc.vector.tensor_scalar(mw[1], wsb[:, 1, 0], 0.0, None, op0=OP.max)
            nc.vector.tensor_scalar(nw[1], wsb[:, 1, 0], 0.0, None, op0=OP.min)
            nc.vector.tensor_tensor(apt[1], mw[1], wsb[:, 1, 1], op=OP.mult)
            nc.vector.tensor_scalar(xn[0], xsb[:, 0], 0.0, None, op0=OP.min)
            nc.vector.tensor_tensor(amt[1], nw[1], wsb[:, 1, 1], op=OP.mult)
            nc.vector.tensor_scalar(xn[1], xsb[:, 1], 0.0, None, op0=OP.min)
            nc.vector.tensor_tensor(am2[1], amt[1][:, 0:2, :], amt[1][:, 2:4, :],
                                    op=OP.add)
            # Pool stream: tile0 pair-adds + tile1's ap pair-add
            nc.gpsimd.tensor_tensor(ap2[0], apt[0][:, 0:2, :], apt[0][:, 2:4, :],
                                    op=OP.add)
            nc.gpsimd.tensor_tensor(am2[0], amt[0][:, 0:2, :], amt[0][:, 2:4, :],
                                    op=OP.add)
            nc.gpsimd.tensor_tensor(ap2[1], apt[1][:, 0:2, :], apt[1][:, 2:4, :],
                                    op=OP.add)
            # ACT stream: relu(x) per chunk, then the epilogue
            for c in range(NC):
                nc.scalar.activation(xp[c], xsb[:, c], AF.Relu)

            # matmuls: one f32 PSUM tile (own bank) per chunk. Emission order
            # is runnable-first: everything gated only by tile-0 prep runs
            # before anything needing tile-1 prep (which lands last), and
            # chunk 0 closes before chunk 1 so its epilogue can start early.
            # matmul groups: chunk 0 full-width, chunk 1 split in half-width
            # psum groups so the tail epilogue is small and overlaps the
            # final matmuls. grp = (psum, xp source c + col slice)
            ps0 = pp.tile([128, CB], f32, name="ps0")
            ps1 = pp.tile([128, CB // 2], f32, name="ps1")
            ps2 = pp.tile([128, CB // 2], f32, name="ps2")
            HB = CB // 2
            grps = [(ps0, 0, slice(0, CB)),
                    (ps1, 1, slice(0, HB)),
                    (ps2, 1, slice(HB, CB))]
            with tc.high_priority():
                for g, c, sl in grps:
                    for p in range(2):
                        nc.tensor.matmul(g, lhsT=ap2[0][:, p, :],
                                         rhs=xp[c][:, 0, sl],
                                         start=(p == 0), stop=False)
                    for p in range(2):
                        nc.tensor.matmul(g, lhsT=am2[0][:, p, :],
                                         rhs=xn[c][:, 0, sl],
                                         start=False, stop=False)
            for g, c, sl in grps:
                for p in range(2):
                    nc.tensor.matmul(g, lhsT=ap2[1][:, p, :],
                                     rhs=xp[c][:, 1, sl],
                                     start=False, stop=False)
                for p in range(2):
                    nc.tensor.matmul(g, lhsT=am2[1][:, p, :],
                                     rhs=xn[c][:, 1, sl],
                                     start=False, stop=(p == 1))

            # fused epilogue: out = S * psum + b2, bf16 — chunk 0 on ACT,
            # chunk-1 halves in parallel on DVE — then one store
            osb = wk.tile([128, BS], bf16)
            nc.scalar.activation(osb[:, 0:CB], ps0,
                                 AF.Identity, bias=b2sb, scale=float(SCALE))
            nc.vector.tensor_scalar(osb[:, CB:CB + HB], ps1, float(SCALE),
                                    b2sb[:, 0:1], op0=OP.mult, op1=OP.add)
            nc.vector.tensor_scalar(osb[:, CB + HB:BS], ps2, float(SCALE),
                                    b2sb[:, 0:1], op0=OP.mult, op1=OP.add)
            nc.sync.dma_start(out=outt[:, :], in_=osb)

    nc.compile()
    _CACHE["nc"] = nc
    return nc


def _kan_numpy(x, w1, b1, w2, b2):
    # exact fallback, chunked over batch to bound memory
    out = np.empty((x.shape[0], w1.shape[0]), dtype=np.float32)
    d = (w1.shape[0] + w1.shape[1]) / 2
    s = 1.0 / np.sqrt(d * w1.shape[2])
    for lo in range(0, x.shape[0], 128):
        hi = min(lo + 128, x.shape[0])
        h = x[lo:hi, None, :, None] * w1[None] + b1[None]
        np.maximum(h, 0.0, out=h)
        out[lo:hi] = np.einsum("boik,oik->bo", h, w2) * s
    return out + b2[None, :]


def kernel(x, w1, b1, w2, b2):
    x = np.ascontiguousarray(x, dtype=np.float32)
    w1 = np.asarray(w1, dtype=np.float32)
    b1 = np.asarray(b1, dtype=np.float32)
    w2 = np.asarray(w2, dtype=np.float32)
    b2 = np.asarray(b2, dtype=np.float32)

    if x.shape != (B, DIN) or w1.shape != (DOUT, DIN, K) or np.any(b1):
        return _kan_numpy(x, w1, b1, w2, b2)

    import ml_dtypes
    from concourse.bass_utils import run_bass_kernel_spmd

    bf16 = ml_dtypes.bfloat16
    nc = _build_nc()

    xT = x.T                                      # (DIN, B)
    w1T = w1.transpose(1, 2, 0)                   # (DIN, K, DOUT)
    w2T = w2.transpose(1, 2, 0)

    in_maps = []
    for core in range(N_CORES):
        bi, oj = divmod(core, OG)
        wpk = np.empty((128, NT, 2, K, OS), dtype=bf16)
        xpk = np.empty((128, NC, NT, CB), dtype=bf16)
        osl = slice(oj * OS, (oj + 1) * OS)
        for t in range(NT):
            isl = slice(t * 128, (t + 1) * 128)
            wpk[:, t, 0] = w1T[isl, :, osl]
            wpk[:, t, 1] = w2T[isl, :, osl]
            for c in range(NC):
                bsl = slice(bi * BS + c * CB, bi * BS + (c + 1) * CB)
                xpk[:, c, t, :] = xT[isl, bsl]
        in_maps.append({
            "wpk": wpk,
            "xpk": xpk,
            "b2s": np.ascontiguousarray(b2[osl]).reshape(OS, 1),
        })

    res = run_bass_kernel_spmd(nc, in_maps, core_ids=list(range(N_CORES)))

    out = np.empty((B, DOUT), dtype=np.float32)
    for core in range(N_CORES):
        bi, oj = divmod(core, OG)
        ot = np.asarray(res.results[core]["outt"]).astype(np.float32)
        out[bi * BS:(bi + 1) * BS, oj * OS:(oj + 1) * OS] = ot.T
    return out
